# revision 11
# baseline (speedup 1.0000x reference)
"""Gemma3-style sliding-window attention on 8 Trainium2 NeuronCores.

Sharding: tensor-parallel over the 8 query heads (1 head per core, KV head
h//2 shared per pair). The K/V projections are split by dimension halves
across each core pair: core 2k computes dims [0,128) of its KV head's K and
V, core 2k+1 computes dims [128,256); the raw halves are exchanged with a
pair-wise AllGather per s-block (DRAM bounce). Each core then applies
k-norm/RoPE locally on the gathered full-D raw K. Each core computes its
head's partial o-proj output [S, HID]; the host sums the 8 partials.

All matmul operands are bf16 (rel-err budget 2e-2); accumulation is f32 in
PSUM. Everything the device consumes transposed is pre-transposed on the
host, so the device issues only natural-layout matmuls.

Softmax trick: scores are softcapped by 50*tanh(.), so they are bounded in
[-50, 50] and exp() never overflows f32 -> no running-max subtraction.
Masking is additive (-2000) on the tanh output (pre-exp), which makes
masked exp() terms exactly 0.
"""

import os
import sys
import types

import numpy as np
import ml_dtypes

BF16 = ml_dtypes.bfloat16

B, S, HID = 1, 2048, 2560
H, KV, D = 8, 4, 256
SCALE = 256 ** -0.5
SOFTCAP = 50.0
WINDOW = 512
EPS = 1e-6
NCORES = 8
DH = 2                 # 128-partition halves of D
NHID = HID // 128      # 20
SB = 512               # s-block size
NSB = S // SB          # 4
NBLK = S // 128        # 16 query blocks
MAXW = WINDOW + 128    # max key span per query block

_COMPILED = None
LAST_RESULT = None     # BassKernelResults of the most recent run (for test.py)


def _install_ntff_shim():
    """The image's antenv lacks axon_hooks; recreate it so trace=True works."""
    try:
        from antenv import axon_hooks  # noqa: F401
        return
    except ImportError:
        pass
    try:
        import antenv
        import trn_agent_boot.trn_boot as tb

        hook = tb._ntff_profile_via_ctypes("/opt/axon/libaxon_pjrt.so")
        mod = types.ModuleType("antenv.axon_hooks")
        mod._hook = hook
        mod.get_axon_ntff_profile_hook = lambda: mod._hook
        mod.set_axon_ntff_profile_hook = lambda h: setattr(mod, "_hook", h)
        sys.modules["antenv.axon_hooks"] = mod
        antenv.axon_hooks = mod
    except Exception:
        pass


def _build():
    import concourse.mybir as mybir
    import concourse.tile as tile
    from concourse import bacc
    from concourse.mybir import ActivationFunctionType as AF

    f32 = mybir.dt.float32
    bf16 = mybir.dt.bfloat16

    nc = bacc.Bacc("TRN2", target_bir_lowering=False, debug=False,
                   num_devices=NCORES)

    hsT_d = nc.dram_tensor("hsT", [128, NHID * S], bf16, kind="ExternalInput")
    wqT_d = nc.dram_tensor("wqT", [128, NHID * D], bf16, kind="ExternalInput")
    # my d-half of the pair's K / V projection weights
    wkT_d = nc.dram_tensor("wkT", [128, NHID * 128], bf16,
                           kind="ExternalInput")
    wvT_d = nc.dram_tensor("wvT", [128, NHID * 128], bf16,
                           kind="ExternalInput")
    woT_d = nc.dram_tensor("woT", [D, HID], bf16, kind="ExternalInput")
    cosT_d = nc.dram_tensor("cosT", [D, S], bf16, kind="ExternalInput")
    sinT_d = nc.dram_tensor("sinT", [D, S], bf16, kind="ExternalInput")
    wtq_d = nc.dram_tensor("wtq", [D, 1], f32, kind="ExternalInput")
    wtk_d = nc.dram_tensor("wtk", [D, 1], f32, kind="ExternalInput")
    mask_d = nc.dram_tensor("maskadd", [128, MAXW], f32, kind="ExternalInput")
    id_d = nc.dram_tensor("ident", [128, 128], bf16, kind="ExternalInput")
    ones_d = nc.dram_tensor("ones_", [128, 1], bf16, kind="ExternalInput")
    out_d = nc.dram_tensor("out", [S, HID], bf16, kind="ExternalOutput")

    C2 = (SOFTCAP / SCALE) ** 2  # folds SCALE/SOFTCAP into the k-norm scale
    GROUPS = [[2 * k, 2 * k + 1] for k in range(NCORES // 2)]
    CCW = 1024           # bounce cols: [0:512] K half, [512:1024] V half

    with tile.TileContext(nc) as tc:
        with tc.tile_pool(name="const", bufs=1) as cp, \
             tc.tile_pool(name="hstp", bufs=2) as hstp, \
             tc.tile_pool(name="evp", bufs=3) as evp, \
             tc.tile_pool(name="smp", bufs=2) as smp, \
             tc.tile_pool(name="dram", bufs=2, space="DRAM") as dp, \
             tc.tile_pool(name="psA", bufs=3, space="PSUM") as psA, \
             tc.tile_pool(name="psS1", bufs=2, space="PSUM") as psS1, \
             tc.tile_pool(name="psT", bufs=2, space="PSUM") as psT, \
             tc.tile_pool(name="psP", bufs=1, space="PSUM") as psP:

            # ---- persistent constants ----
            wq_sb = cp.tile([128, NHID * D], bf16, tag="wq", name="wq")
            wk_sb = cp.tile([128, NHID * 128], bf16, tag="wk", name="wk")
            wv_sb = cp.tile([128, NHID * 128], bf16, tag="wv", name="wv")
            wo_sb = [cp.tile([128, HID], bf16, tag=f"wo{d}", name=f"wo{d}")
                     for d in range(DH)]
            cos_sb = [cp.tile([128, S], bf16, tag=f"cos{d}", name=f"cos{d}")
                      for d in range(DH)]
            sin_sb = [cp.tile([128, S], bf16, tag=f"sin{d}", name=f"sin{d}")
                      for d in range(DH)]
            wtq_sb = [cp.tile([128, 1], f32, tag=f"wtq{d}", name=f"wtq{d}")
                      for d in range(DH)]
            wtk_sb = [cp.tile([128, 1], f32, tag=f"wtk{d}", name=f"wtk{d}")
                      for d in range(DH)]
            mask_sb = cp.tile([128, MAXW], f32, tag="mask", name="mask")
            id_sb = cp.tile([128, 128], bf16, tag="ident", name="ident")
            ones_sb = cp.tile([128, 1], bf16, tag="ones", name="ones")
            epsq_sb = cp.tile([128, 1], f32, tag="epsq", name="epsq")
            epsk_sb = cp.tile([128, 1], f32, tag="epsk", name="epsk")
            nc.vector.memset(epsq_sb, EPS)
            nc.vector.memset(epsk_sb, C2 * EPS)

            # persistent activations
            qwT = [cp.tile([128, S], bf16, tag=f"qwT{d}", name=f"qwT{d}")
                   for d in range(DH)]
            kwT = [cp.tile([128, S], bf16, tag=f"kwT{d}", name=f"kwT{d}")
                   for d in range(DH)]
            outT = [cp.tile([128, S], bf16, tag=f"outT{d}", name=f"outT{d}")
                    for d in range(DH)]
            v_sb = [cp.tile([128, D], bf16, tag=f"v{m}", name=f"v{m}")
                    for m in range(NBLK)]
            beta_bc = cp.tile([128, S], f32, tag="betabc", name="betabc")
            alpha = cp.tile([128, NBLK], f32, tag="alpha", name="alpha")
            dn = cp.tile([128, NBLK], f32, tag="dn", name="dn")
            rc = cp.tile([128, NBLK], f32, tag="rc", name="rc")

            # warmup collective: absorbs the ~30us ncfw entry latency while
            # the input DMAs stream
            wu_in = dp.tile([1, 16], bf16, tag="wui", name="wu_in")
            wu_out = dp.tile([2, 16], bf16, tag="wuo", name="wu_out")
            nc.sync.dma_start(wu_in[:], id_d.ap()[0:1, 0:16])
            nc.gpsimd.collective_compute(
                "AllGather", mybir.AluOpType.bypass, replica_groups=GROUPS,
                ins=[wu_in.opt()], outs=[wu_out.opt()])

            # piecewise wide-line loads, interleaved so the first
            # projection matmuls start after the first piece lands
            SBW = NHID * SB      # columns per s-block in the streamed layout
            hst0 = hstp.tile([128, SBW], bf16, tag="hstbig", name="hst_sb0")
            pieces = [1, 2, 2, 5, 5, 5]   # chunk counts; small first pieces
            c0 = 0
            for cnt in pieces:
                wsl = slice(c0 * D, (c0 + cnt) * D)
                hsl2 = slice(c0 * 128, (c0 + cnt) * 128)
                nc.sync.dma_start(wq_sb[:, wsl], wqT_d.ap()[:, wsl])
                nc.sync.dma_start(wk_sb[:, hsl2], wkT_d.ap()[:, hsl2])
                hsl = slice(c0 * SB, (c0 + cnt) * SB)
                nc.sync.dma_start(hst0[:, hsl], hsT_d.ap()[:, hsl])
                c0 += cnt
            nc.sync.dma_start(wv_sb, wvT_d.ap())
            # needed from RoPE / softmax onward — after the first projection
            for d in range(DH):
                r = slice(d * 128, (d + 1) * 128)
                nc.sync.dma_start(cos_sb[d], cosT_d.ap()[r, :])
                nc.sync.dma_start(sin_sb[d], sinT_d.ap()[r, :])
                nc.sync.dma_start(wtq_sb[d], wtq_d.ap()[r, :])
                nc.sync.dma_start(wtk_sb[d], wtk_d.ap()[r, :])
            nc.sync.dma_start(mask_sb, mask_d.ap())
            nc.sync.dma_start(id_sb, id_d.ap())
            nc.sync.dma_start(ones_sb, ones_d.ap())
            # needed only at o-proj
            for d in range(DH):
                r = slice(d * 128, (d + 1) * 128)
                nc.sync.dma_start(wo_sb[d], woT_d.ap()[r, :])

            def emit_oproj_block(b):
                    qsl = slice(b * 128, (b + 1) * 128)
                    orow = smp.tile([128, HID], bf16, tag="orow", bufs=3,
                                    name=f"orow{b}")
                    for n in range(5):
                        op = psA.tile([128, SB], f32, tag="acc",
                                      name=f"op{b}_{n}")
                        nsl = slice(n * 512, (n + 1) * 512)
                        for d in range(DH):
                            nc.tensor.matmul(op, outT[d][:, qsl],
                                             wo_sb[d][:, nsl],
                                             start=(d == 0),
                                             stop=(d == DH - 1))
                        if n % 2 == 0:
                            nc.scalar.mul(orow[:, nsl], op, rc[:, b:b + 1])
                        else:
                            nc.vector.tensor_scalar_mul(orow[:, nsl], op,
                                                        rc[:, b:b + 1])
                        if b == NBLK - 1:
                            # last block: per-chunk DMA so the tail drains
                            # as each eviction completes
                            nc.sync.dma_start(
                                out_d.ap()[b * 128:(b + 1) * 128, nsl],
                                orow[:, nsl])
                    if b != NBLK - 1:
                        nc.sync.dma_start(
                            out_d.ap()[b * 128:(b + 1) * 128, :], orow)

            hst_tiles = []
            for sb in range(NSB):
                s0 = sb * SB
                sl = slice(s0, s0 + SB)

                # ---- hidden-state stream for this s-block ----
                if sb == 0:
                    hst_tiles.append(hst0)
                if sb + 1 < NSB:
                    nxt = hstp.tile([128, SBW], bf16, tag="hstbig",
                                    name=f"hst_sb{sb + 1}")
                    nc.sync.dma_start(
                        nxt, hsT_d.ap()[:, (sb + 1) * SBW:(sb + 2) * SBW])
                    hst_tiles.append(nxt)
                hstb = hst_tiles[sb]

                # ---- q projection; RoPE fused with the (1+w) scale reads
                # the projection PSUM directly via scalar_tensor_tensor ----
                qsq = [evp.tile([128, SB], bf16, tag=f"qsq{d}",
                                name=f"qsq{sb}_{d}") for d in range(DH)]
                MUL = mybir.AluOpType.mult

                def rope_combine(srcs, wtt, dstT, tag, beta=None):
                    # srcs[d]: raw projection half d ([128, SB], PSUM f32 or
                    # SBUF bf16)
                    ra = smp.tile([128, SB], bf16, tag="ra", bufs=3,
                                  name=f"ra{tag}{sb}")
                    rb = smp.tile([128, SB], bf16, tag="rb", bufs=3,
                                  name=f"rb{tag}{sb}")
                    nc.vector.scalar_tensor_tensor(
                        ra, srcs[0], wtt[0], cos_sb[0][:, sl], MUL, MUL)
                    nc.vector.scalar_tensor_tensor(
                        rb, srcs[1], wtt[1], sin_sb[0][:, sl], MUL, MUL)
                    if beta is None:
                        nc.vector.tensor_sub(dstT[0][:, sl], ra, rb)
                    else:
                        t0b = smp.tile([128, SB], bf16, tag="ra", bufs=3,
                                       name=f"t0{tag}{sb}")
                        nc.vector.tensor_sub(t0b, ra, rb)
                        nc.vector.tensor_mul(dstT[0][:, sl], t0b,
                                             beta[:, sl])
                    rc2 = smp.tile([128, SB], bf16, tag="ra", bufs=3,
                                   name=f"rc{tag}{sb}")
                    rd = smp.tile([128, SB], bf16, tag="rb", bufs=3,
                                  name=f"rd{tag}{sb}")
                    nc.vector.scalar_tensor_tensor(
                        rc2, srcs[1], wtt[1], cos_sb[1][:, sl], MUL, MUL)
                    nc.vector.scalar_tensor_tensor(
                        rd, srcs[0], wtt[0], sin_sb[1][:, sl], MUL, MUL)
                    if beta is None:
                        nc.vector.tensor_add(dstT[1][:, sl], rc2, rd)
                    else:
                        t1b = smp.tile([128, SB], bf16, tag="rb", bufs=3,
                                       name=f"t1{tag}{sb}")
                        nc.vector.tensor_add(t1b, rc2, rd)
                        nc.vector.tensor_mul(dstT[1][:, sl], t1b,
                                             beta[:, sl])

                # Q projection (both halves local) + rope-q
                qpps = []
                for d in range(DH):
                    pp = psA.tile([128, SB], f32, tag="acc",
                                  name=f"ppq{sb}_{d}")
                    for t in range(NHID):
                        lsl = slice(t * D + d * 128, t * D + (d + 1) * 128)
                        nc.tensor.matmul(pp, wq_sb[:, lsl],
                                         hstb[:, t * SB:(t + 1) * SB],
                                         start=(t == 0), stop=(t == NHID - 1))
                    nc.scalar.activation(qsq[d], pp, AF.Square)
                    qpps.append(pp)
                rope_combine([qpps[0][:, :], qpps[1][:, :]], wtq_sb, qwT, "q")

                # ---- my K d-half: raw [128 dims, SB] ----
                kp = psA.tile([128, SB], f32, tag="acc", name=f"kp{sb}")
                for t in range(NHID):
                    nc.tensor.matmul(kp, wk_sb[:, t * 128:(t + 1) * 128],
                                     hstb[:, t * SB:(t + 1) * SB],
                                     start=(t == 0), stop=(t == NHID - 1))
                kraw_my = smp.tile([128, SB], bf16, tag="krawm", bufs=2,
                                   name=f"krawm{sb}")
                nc.vector.tensor_copy(kraw_my, kp)

                # ---- my V d-half: raw [s, 128] per 128-row m-block ----
                vraw_my = []
                for m in range(4):
                    vp = psP.tile([128, 128], f32, tag="pp",
                                  name=f"vp{sb}_{m}")
                    for t in range(NHID):
                        msl = slice(t * SB + m * 128, t * SB + (m + 1) * 128)
                        nc.tensor.matmul(vp, hstb[:, msl],
                                         wv_sb[:, t * 128:(t + 1) * 128],
                                         start=(t == 0), stop=(t == NHID - 1))
                    vr = smp.tile([128, 128], bf16, tag="vraw", bufs=8,
                                  name=f"vraw{sb}_{m}")
                    nc.vector.tensor_copy(vr, vp)
                    vraw_my.append(vr)

                # ---- pair exchange of the raw halves ----
                cc_in = dp.tile([128, CCW], bf16, tag="ccin",
                                name=f"ccin{sb}")
                cc_out = dp.tile([256, CCW], bf16, tag="ccout",
                                 name=f"ccout{sb}")
                nc.sync.dma_start(cc_in[:, 0:SB], kraw_my)
                for m in range(4):
                    nc.sync.dma_start(
                        cc_in[:, SB + m * 128:SB + (m + 1) * 128],
                        vraw_my[m])
                nc.gpsimd.collective_compute(
                    "AllGather", mybir.AluOpType.bypass,
                    replica_groups=GROUPS,
                    ins=[cc_in.opt()], outs=[cc_out.opt()])
                kraw = [smp.tile([128, SB], bf16, tag=f"kraw{d}", bufs=2,
                                 name=f"kraw{sb}_{d}") for d in range(DH)]
                nc.sync.dma_start(kraw[0], cc_out[0:128, 0:SB])
                nc.sync.dma_start(kraw[1], cc_out[128:256, 0:SB])
                for m in range(4):
                    csl = slice(SB + m * 128, SB + (m + 1) * 128)
                    nc.sync.dma_start(v_sb[sb * 4 + m][:, 0:128],
                                      cc_out[0:128, csl])
                    nc.sync.dma_start(v_sb[sb * 4 + m][:, 128:256],
                                      cc_out[128:256, csl])

                # previous s-block's o-proj: PE filler during the exchange
                if sb > 0:
                    for mm in range(4):
                        emit_oproj_block((sb - 1) * 4 + mm)

                # ---- alpha = 1/sqrt(mean(q^2)+eps), per query row ----
                ap_ps = psP.tile([128, 4], f32, tag="pp",
                                 name=f"aps{sb}")
                for m in range(4):
                    msl = slice(m * 128, (m + 1) * 128)
                    for d in range(DH):
                        nc.tensor.matmul(ap_ps[:, m:m + 1], qsq[d][:, msl],
                                         ones_sb, start=(d == 0),
                                         stop=(d == DH - 1))
                atmp = smp.tile([128, 4], f32, tag="atmp", name=f"atmp{sb}")
                nc.scalar.activation(atmp, ap_ps, AF.Sqrt,
                                     bias=epsq_sb, scale=1.0 / D)
                nc.vector.reciprocal_approx_fast(
                    alpha[:, sb * 4:(sb + 1) * 4], atmp)

                # ---- beta from the gathered K halves ----
                ksq = [evp.tile([128, SB], bf16, tag=f"ksq{d}",
                                name=f"ksq{sb}_{d}") for d in range(DH)]
                for d in range(DH):
                    nc.scalar.activation(ksq[d], kraw[d], AF.Square)
                bp = psS1.tile([1, SB], f32, tag="sc1", name=f"bp{sb}")
                for d in range(DH):
                    nc.tensor.matmul(bp, ones_sb, ksq[d],
                                     start=(d == 0), stop=(d == DH - 1))
                btmp = smp.tile([1, SB], f32, tag="btmp", name=f"btmp{sb}")
                nc.scalar.activation(btmp, bp, AF.Sqrt,
                                     bias=epsk_sb[0:1, :], scale=C2 / D)
                brow = smp.tile([1, SB], f32, tag="brow", name=f"brow{sb}")
                nc.vector.reciprocal_approx_fast(brow, btmp)
                nc.gpsimd.partition_broadcast(beta_bc[:, sl], brow)

                # ---- rope-k on the gathered raw halves ----
                rope_combine([kraw[0][:, :], kraw[1][:, :]], wtk_sb, kwT,
                             "k", beta=beta_bc)

                # ---- attention + o-proj per 128-row query block ----
                for m in range(4):
                    b = sb * 4 + m
                    q0 = b * 128
                    w = min(b + 1, 5) * 128
                    k0 = q0 + 128 - w
                    w1 = w - 128
                    qsl = slice(q0, q0 + 128)

                    if w1 > 0:
                        sc1 = psS1.tile([128, SB], f32, tag="sc1",
                                        name=f"sc1_{b}")
                        for d in range(DH):
                            nc.tensor.matmul(sc1[:, 0:w1], qwT[d][:, qsl],
                                             kwT[d][:, k0:k0 + w1],
                                             start=(d == 0),
                                             stop=(d == DH - 1))
                    sc2 = psT.tile([128, 128], f32, tag="tp",
                                   name=f"sc2_{b}")
                    for d in range(DH):
                        nc.tensor.matmul(sc2, qwT[d][:, qsl], kwT[d][:, qsl],
                                         start=(d == 0), stop=(d == DH - 1))

                    tt = smp.tile([128, MAXW], f32, tag="tanh", bufs=3,
                                  name=f"tt{b}")
                    if w1 > 0:
                        nc.scalar.activation(tt[:, 0:w1], sc1[:, 0:w1],
                                             AF.Tanh, scale=alpha[:, b:b + 1])
                    nc.scalar.activation(tt[:, w1:w], sc2, AF.Tanh,
                                         scale=alpha[:, b:b + 1])
                    nc.vector.tensor_add(tt[:, w1:w], tt[:, w1:w],
                                         mask_sb[:, 512:640])
                    if w == MAXW:
                        nc.vector.tensor_add(tt[:, 0:128], tt[:, 0:128],
                                             mask_sb[:, 0:128])

                    et = smp.tile([128, MAXW], bf16, tag="et", bufs=4,
                                  name=f"et{b}")
                    nc.scalar.activation(et[:, 0:w], tt[:, 0:w], AF.Exp,
                                         scale=SOFTCAP,
                                         accum_out=dn[:, b:b + 1])
                    # 1/denominator is applied at the o-proj eviction (rows
                    # of that PSUM are queries), keeping it off the softmax
                    # critical path
                    nc.vector.reciprocal_approx_fast(rc[:, b:b + 1],
                                                     dn[:, b:b + 1])

                    nchunks = w // 128
                    etcs = []
                    for c in range(nchunks):
                        tp = psT.tile([128, 128], bf16, tag="tp",
                                      name=f"tp{b}_{c}")
                        nc.tensor.transpose(tp, et[:, c * 128:(c + 1) * 128],
                                            id_sb)
                        etc = smp.tile([128, 128], bf16, tag="etc", bufs=6,
                                       name=f"etc{b}_{c}")
                        nc.vector.tensor_copy(etc, tp)
                        etcs.append(etc)
                    po = psP.tile([128, D], f32, tag="pp", name=f"po{b}")
                    for d in range(DH):
                        dsl = slice(d * 128, (d + 1) * 128)
                        for c in range(nchunks):
                            kvi = k0 // 128 + c
                            nc.tensor.matmul(po[:, dsl], v_sb[kvi][:, dsl],
                                             etcs[c], start=(c == 0),
                                             stop=(c == nchunks - 1))
                    for d in range(DH):
                        dsl = slice(d * 128, (d + 1) * 128)
                        nc.vector.tensor_copy(outT[d][:, qsl], po[:, dsl])

                    if sb == NSB - 1:
                        emit_oproj_block(b)

    nc.compile()
    return nc


def _prep_in_maps(hidden_states, position_ids, cos_table, sin_table,
                  Wq, Wk, Wv, Wo, q_norm_w, k_norm_w):
    hs = np.asarray(hidden_states, np.float32).reshape(S, HID)
    pos = np.asarray(position_ids).reshape(S).astype(np.int64)
    cos = np.asarray(cos_table, np.float32)[pos]   # [S, D]
    sin = np.asarray(sin_table, np.float32)[pos]
    Wq = np.asarray(Wq, np.float32)
    Wk = np.asarray(Wk, np.float32)
    Wv = np.asarray(Wv, np.float32)
    Wo = np.asarray(Wo, np.float32)

    # streamed layout: [128, sb*(NHID*SB) + t*SB + s'] so every DMA line is
    # wide and contiguous
    hsT = np.ascontiguousarray(
        hs.T.astype(BF16).reshape(NHID, 128, NSB, SB)
        .transpose(1, 2, 0, 3).reshape(128, NHID * S))
    cosT = np.ascontiguousarray(cos.T).astype(BF16)
    sinT = np.ascontiguousarray(sin.T).astype(BF16)

    def wtile(wslice):
        # [Dout, HID] -> [128, NHID*Dout] with chunk t at cols [t*Dout, ...)
        dout = wslice.shape[0]
        return np.ascontiguousarray(
            wslice.T.astype(BF16).reshape(NHID, 128, dout)
            .transpose(1, 0, 2).reshape(128, NHID * dout))
    wtq = (1.0 + np.asarray(q_norm_w, np.float32)).reshape(D, 1)
    wtk = (1.0 + np.asarray(k_norm_w, np.float32)).reshape(D, 1)

    i = np.arange(128)[:, None]
    j = np.arange(128)[None, :]
    mask = np.zeros((128, MAXW), np.float32)
    mask[:, 0:128] = np.where(j > i, 0.0, -2000.0)      # oldest chunk
    mask[:, 512:640] = np.where(j <= i, 0.0, -2000.0)   # causal chunk
    ident = np.eye(128, dtype=BF16)
    ones = np.ones((128, 1), BF16)

    in_maps = []
    for h in range(NCORES):
        kv = h // (H // KV)
        dh = h % 2
        qs = slice(h * D, (h + 1) * D)
        khs = slice(kv * D + dh * 128, kv * D + (dh + 1) * 128)
        in_maps.append({
            "hsT": hsT,
            "wqT": wtile(Wq[qs, :]),
            "wkT": wtile(Wk[khs, :]),
            "wvT": wtile(Wv[khs, :]),
            "woT": np.ascontiguousarray(Wo[:, qs].T).astype(BF16),
            "cosT": cosT, "sinT": sinT,
            "wtq": wtq, "wtk": wtk,
            "maskadd": mask, "ident": ident, "ones_": ones,
        })
    return in_maps


def kernel(hidden_states, position_ids, cos_table, sin_table,
           Wq, Wk, Wv, Wo, q_norm_w, k_norm_w):
    global _COMPILED, LAST_RESULT
    trace = bool(os.environ.get("BASS_TRACE"))
    if trace:
        _install_ntff_shim()
    from concourse import bass_utils

    if _COMPILED is None:
        _COMPILED = _build()

    in_maps = _prep_in_maps(hidden_states, position_ids, cos_table,
                            sin_table, Wq, Wk, Wv, Wo, q_norm_w, k_norm_w)
    res = bass_utils.run_bass_kernel_spmd(
        _COMPILED, in_maps, core_ids=list(range(NCORES)), trace=trace)
    LAST_RESULT = res

    out = res.results[0]["out"].astype(np.float32)
    for i in range(1, NCORES):
        out += res.results[i]["out"]
    return out.reshape(B, S, HID)


# revision 19
# speedup vs baseline: 1.1570x; 1.1570x over previous
"""Gemma3-style sliding-window attention on 8 Trainium2 NeuronCores.

Sharding: tensor-parallel over the 8 query heads (1 head per core, KV head
h//2 shared per pair). The K/V projections are split by dimension halves
across each core pair: core 2k computes dims [0,128) of its KV head's K and
V, core 2k+1 computes dims [128,256); the raw halves (plus the per-half
k^2 column sums) are exchanged with a pair-wise AllGather per s-block
(DRAM bounce, SWDGE so the bounce DMAs bypass the loaded HWDGE queues).
Each core then applies k-norm/RoPE locally on the gathered full-D raw K.
K/V for s-block j+1 are produced during iteration j so the exchange
latency hides behind local compute. A tiny warmup collective issued at
kernel start absorbs the ~45us ncfw entry latency.

Each core computes its head's partial o-proj output [S, HID]; the host
sums the 8 partials. All matmul operands are bf16 (rel-err budget 2e-2);
accumulation is f32 in PSUM.

Softmax trick: scores are softcapped by 50*tanh(.), so they are bounded in
[-50, 50] and exp() never overflows f32 -> no running-max subtraction.
Masking is additive (-2000) on the tanh output (pre-exp), which makes
masked exp() terms exactly 0.
"""

import os
import sys
import types

import numpy as np
import ml_dtypes

BF16 = ml_dtypes.bfloat16

B, S, HID = 1, 2048, 2560
H, KV, D = 8, 4, 256
SCALE = 256 ** -0.5
SOFTCAP = 50.0
WINDOW = 512
EPS = 1e-6
NCORES = 8
DH = 2                 # 128-partition halves of D
NHID = HID // 128      # 20
SB = 512               # s-block size
NSB = S // SB          # 4
NBLK = S // 128        # 16 query blocks
MAXW = WINDOW + 128    # max key span per query block

_COMPILED = None
LAST_RESULT = None     # BassKernelResults of the most recent run (for test.py)


def _install_ntff_shim():
    """The image's antenv lacks axon_hooks; recreate it so trace=True works."""
    try:
        from antenv import axon_hooks  # noqa: F401
        return
    except ImportError:
        pass
    try:
        import antenv
        import trn_agent_boot.trn_boot as tb

        hook = tb._ntff_profile_via_ctypes("/opt/axon/libaxon_pjrt.so")
        mod = types.ModuleType("antenv.axon_hooks")
        mod._hook = hook
        mod.get_axon_ntff_profile_hook = lambda: mod._hook
        mod.set_axon_ntff_profile_hook = lambda h: setattr(mod, "_hook", h)
        sys.modules["antenv.axon_hooks"] = mod
        antenv.axon_hooks = mod
    except Exception:
        pass


def _build():
    import concourse.mybir as mybir
    import concourse.tile as tile
    from concourse import bacc
    from concourse.mybir import ActivationFunctionType as AF

    f32 = mybir.dt.float32
    bf16 = mybir.dt.bfloat16

    nc = bacc.Bacc("TRN2", target_bir_lowering=False, debug=False,
                   num_devices=NCORES)

    hsT_d = nc.dram_tensor("hsT", [128, NHID * S], bf16, kind="ExternalInput")
    wqT_d = nc.dram_tensor("wqT", [128, NHID * D], bf16, kind="ExternalInput")
    # my d-half of the pair's K / V projection weights
    wkT_d = nc.dram_tensor("wkT", [128, NHID * 128], bf16,
                           kind="ExternalInput")
    wvT_d = nc.dram_tensor("wvT", [128, NHID * 128], bf16,
                           kind="ExternalInput")
    woT_d = nc.dram_tensor("woT", [D, HID], bf16, kind="ExternalInput")
    cosT_d = nc.dram_tensor("cosT", [D, S], bf16, kind="ExternalInput")
    sinT_d = nc.dram_tensor("sinT", [D, S], bf16, kind="ExternalInput")
    wtq_d = nc.dram_tensor("wtq", [D, 1], f32, kind="ExternalInput")
    wtk_d = nc.dram_tensor("wtk", [D, 1], f32, kind="ExternalInput")
    mask_d = nc.dram_tensor("maskadd", [128, MAXW], f32, kind="ExternalInput")
    id_d = nc.dram_tensor("ident", [128, 128], bf16, kind="ExternalInput")
    ones_d = nc.dram_tensor("ones_", [128, 1], bf16, kind="ExternalInput")
    out_d = nc.dram_tensor("out", [S, HID], bf16, kind="ExternalOutput")

    C2 = (SOFTCAP / SCALE) ** 2  # folds SCALE/SOFTCAP into the k-norm scale
    GROUPS = [[2 * k, 2 * k + 1] for k in range(NCORES // 2)]
    CCW = 1028     # bounce cols: [0:512] K, [512:1024] V, [1024:1028] bp

    with tile.TileContext(nc) as tc:
        with tc.tile_pool(name="const", bufs=1) as cp, \
             tc.tile_pool(name="hstp", bufs=3) as hstp, \
             tc.tile_pool(name="evp", bufs=3) as evp, \
             tc.tile_pool(name="smp", bufs=2) as smp, \
             tc.tile_pool(name="dram", bufs=2, space="DRAM") as dp, \
             tc.tile_pool(name="psA", bufs=3, space="PSUM") as psA, \
             tc.tile_pool(name="psS1", bufs=2, space="PSUM") as psS1, \
             tc.tile_pool(name="psT", bufs=2, space="PSUM") as psT, \
             tc.tile_pool(name="psP", bufs=1, space="PSUM") as psP:

            # ---- persistent constants ----
            wq_sb = cp.tile([128, NHID * D], bf16, tag="wq", name="wq")
            wk_sb = cp.tile([128, NHID * 128], bf16, tag="wk", name="wk")
            wv_sb = cp.tile([128, NHID * 128], bf16, tag="wv", name="wv")
            wo_sb = [cp.tile([128, HID], bf16, tag=f"wo{d}", name=f"wo{d}")
                     for d in range(DH)]
            cos_sb = [cp.tile([128, S], bf16, tag=f"cos{d}", name=f"cos{d}")
                      for d in range(DH)]
            sin_sb = [cp.tile([128, S], bf16, tag=f"sin{d}", name=f"sin{d}")
                      for d in range(DH)]
            wtq_sb = [cp.tile([128, 1], f32, tag=f"wtq{d}", name=f"wtq{d}")
                      for d in range(DH)]
            wtk_sb = [cp.tile([128, 1], f32, tag=f"wtk{d}", name=f"wtk{d}")
                      for d in range(DH)]
            mask_sb = cp.tile([128, MAXW], f32, tag="mask", name="mask")
            id_sb = cp.tile([128, 128], bf16, tag="ident", name="ident")
            ones_sb = cp.tile([128, 1], bf16, tag="ones", name="ones")
            epsq_sb = cp.tile([128, 1], f32, tag="epsq", name="epsq")
            epsk_sb = cp.tile([1, 1], f32, tag="epsk", name="epsk")
            nc.vector.memset(epsq_sb, EPS)
            nc.vector.memset(epsk_sb, C2 * EPS)

            # persistent activations
            qwT = [cp.tile([128, S], bf16, tag=f"qwT{d}", name=f"qwT{d}")
                   for d in range(DH)]
            kwT = [cp.tile([128, S], bf16, tag=f"kwT{d}", name=f"kwT{d}")
                   for d in range(DH)]
            outT = [cp.tile([128, S], bf16, tag=f"outT{d}", name=f"outT{d}")
                    for d in range(DH)]
            v_sb = [cp.tile([128, D], bf16, tag=f"v{m}", name=f"v{m}")
                    for m in range(NBLK)]
            beta_bc = cp.tile([128, S], bf16, tag="betabc", name="betabc")
            alpha = cp.tile([128, NBLK], f32, tag="alpha", name="alpha")
            dn = cp.tile([128, NBLK], f32, tag="dn", name="dn")
            rc = cp.tile([128, NBLK], f32, tag="rc", name="rc")

            # warmup collective: absorbs the ~45us ncfw entry latency while
            # the input DMAs stream (SWDGE so it triggers immediately)
            wu_in = dp.tile([1, 16], bf16, tag="wui", name="wu_in")
            wu_out = dp.tile([2, 16], bf16, tag="wuo", name="wu_out")
            nc.gpsimd.dma_start(wu_in[:], id_d.ap()[0:1, 0:16])
            nc.gpsimd.collective_compute(
                "AllGather", mybir.AluOpType.bypass, replica_groups=GROUPS,
                ins=[wu_in.opt()], outs=[wu_out.opt()])

            # piecewise wide-line loads, interleaved so the first
            # projection matmuls start after the first piece lands
            SBW = NHID * SB      # columns per s-block in the streamed layout
            hst_tiles = [hstp.tile([128, SBW], bf16, tag="hstbig",
                                   name=f"hst_sb{j}") for j in range(NSB)]
            pieces = [1, 2, 2, 5, 5, 5]   # chunk counts; small first pieces
            c0 = 0
            for cnt in pieces:
                wsl = slice(c0 * D, (c0 + cnt) * D)
                hsl2 = slice(c0 * 128, (c0 + cnt) * 128)
                nc.sync.dma_start(wk_sb[:, hsl2], wkT_d.ap()[:, hsl2])
                nc.sync.dma_start(wq_sb[:, wsl], wqT_d.ap()[:, wsl])
                hsl = slice(c0 * SB, (c0 + cnt) * SB)
                nc.sync.dma_start(hst_tiles[0][:, hsl], hsT_d.ap()[:, hsl])
                c0 += cnt
            nc.sync.dma_start(wv_sb, wvT_d.ap())
            nc.sync.dma_start(hst_tiles[1], hsT_d.ap()[:, SBW:2 * SBW])
            # cos/sin for s-block 0 first (rope-q of sb0), rest after hst1
            for d in range(DH):
                r = slice(d * 128, (d + 1) * 128)
                nc.sync.dma_start(cos_sb[d][:, 0:SB], cosT_d.ap()[r, 0:SB])
                nc.sync.dma_start(sin_sb[d][:, 0:SB], sinT_d.ap()[r, 0:SB])
                nc.sync.dma_start(wtq_sb[d], wtq_d.ap()[r, :])
                nc.sync.dma_start(wtk_sb[d], wtk_d.ap()[r, :])
            nc.sync.dma_start(mask_sb, mask_d.ap())
            nc.sync.dma_start(id_sb, id_d.ap())
            nc.sync.dma_start(ones_sb, ones_d.ap())
            for d in range(DH):
                r = slice(d * 128, (d + 1) * 128)
                nc.sync.dma_start(cos_sb[d][:, SB:], cosT_d.ap()[r, SB:])
                nc.sync.dma_start(sin_sb[d][:, SB:], sinT_d.ap()[r, SB:])
            # needed only at o-proj
            for d in range(DH):
                r = slice(d * 128, (d + 1) * 128)
                nc.sync.dma_start(wo_sb[d], woT_d.ap()[r, :])

            def emit_oproj_block(b):
                    qsl = slice(b * 128, (b + 1) * 128)
                    orow = smp.tile([128, HID], bf16, tag="orow", bufs=2,
                                    name=f"orow{b}")
                    for n in range(5):
                        op = psA.tile([128, SB], f32, tag="acc",
                                      name=f"op{b}_{n}")
                        nsl = slice(n * 512, (n + 1) * 512)
                        for d in range(DH):
                            nc.tensor.matmul(op, outT[d][:, qsl],
                                             wo_sb[d][:, nsl],
                                             start=(d == 0),
                                             stop=(d == DH - 1))
                        if n % 2 == 0:
                            nc.scalar.mul(orow[:, nsl], op, rc[:, b:b + 1])
                        else:
                            nc.vector.tensor_scalar_mul(orow[:, nsl], op,
                                                        rc[:, b:b + 1])
                        if b == NBLK - 1:
                            # last block: per-chunk DMA so the tail drains
                            # as each eviction completes
                            nc.sync.dma_start(
                                out_d.ap()[b * 128:(b + 1) * 128, nsl],
                                orow[:, nsl])
                    if b != NBLK - 1:
                        nc.sync.dma_start(
                            out_d.ap()[b * 128:(b + 1) * 128, :], orow)

            # per-s-block exchange state, filled by kv_produce(j)
            kraw_t = [None] * NSB
            bps_t = [None] * NSB

            def kv_produce(j):
                """Project my K/V d-half of s-block j, issue the pair
                AllGather, and the readbacks into kraw/v_sb/bps."""
                hstj = hst_tiles[j]
                kp = psA.tile([128, SB], f32, tag="acc", name=f"kp{j}")
                for t in range(NHID):
                    nc.tensor.matmul(kp, wk_sb[:, t * 128:(t + 1) * 128],
                                     hstj[:, t * SB:(t + 1) * SB],
                                     start=(t == 0), stop=(t == NHID - 1))
                ksq_my = smp.tile([128, SB], bf16, tag="ksqm", bufs=2,
                                  name=f"ksqm{j}")
                nc.scalar.activation(ksq_my, kp, AF.Square)
                kraw_my = smp.tile([128, SB], bf16, tag="krawm", bufs=2,
                                   name=f"krawm{j}")
                nc.vector.tensor_copy(kraw_my, kp)
                bpp = psS1.tile([1, SB], f32, tag="sc1", name=f"bpp{j}")
                nc.tensor.matmul(bpp, ones_sb, ksq_my, start=True, stop=True)
                bp_my = smp.tile([1, SB], bf16, tag="bpm", bufs=2,
                                 name=f"bpm{j}")
                nc.vector.tensor_copy(bp_my, bpp)

                vraw_my = []
                for m in range(4):
                    vp = psP.tile([128, 128], f32, tag="pp",
                                  name=f"vp{j}_{m}")
                    for t in range(NHID):
                        msl = slice(t * SB + m * 128, t * SB + (m + 1) * 128)
                        nc.tensor.matmul(vp, hstj[:, msl],
                                         wv_sb[:, t * 128:(t + 1) * 128],
                                         start=(t == 0), stop=(t == NHID - 1))
                    vr = smp.tile([128, 128], bf16, tag="vraw", bufs=8,
                                  name=f"vraw{j}_{m}")
                    nc.vector.tensor_copy(vr, vp)
                    vraw_my.append(vr)

                cc_in = dp.tile([128, CCW], bf16, tag="ccin", name=f"cci{j}")
                cc_out = dp.tile([256, CCW], bf16, tag="ccout",
                                 name=f"cco{j}")
                nc.gpsimd.dma_start(cc_in[:, 0:SB], kraw_my)
                for m in range(4):
                    nc.gpsimd.dma_start(
                        cc_in[:, SB + m * 128:SB + (m + 1) * 128],
                        vraw_my[m])
                nc.gpsimd.dma_start(cc_in[:, 1024:1028], bp_my)
                nc.gpsimd.collective_compute(
                    "AllGather", mybir.AluOpType.bypass,
                    replica_groups=GROUPS,
                    ins=[cc_in.opt()], outs=[cc_out.opt()])
                kraw = [smp.tile([128, SB], bf16, tag=f"kraw{d}", bufs=2,
                                 name=f"kraw{j}_{d}") for d in range(DH)]
                nc.sync.dma_start(kraw[0], cc_out[0:128, 0:SB])
                nc.sync.dma_start(kraw[1], cc_out[128:256, 0:SB])
                for m in range(4):
                    csl = slice(SB + m * 128, SB + (m + 1) * 128)
                    nc.sync.dma_start(v_sb[j * 4 + m][:, 0:128],
                                      cc_out[0:128, csl])
                    nc.sync.dma_start(v_sb[j * 4 + m][:, 128:256],
                                      cc_out[128:256, csl])
                bps = [smp.tile([1, SB], bf16, tag=f"bps{d}", bufs=2,
                                name=f"bps{j}_{d}") for d in range(DH)]
                nc.sync.dma_start(bps[0], cc_out[0:128, 1024:1028])
                nc.sync.dma_start(bps[1], cc_out[128:256, 1024:1028])
                kraw_t[j] = kraw
                bps_t[j] = bps

            kv_produce(0)

            for sb in range(NSB):
                s0 = sb * SB
                sl = slice(s0, s0 + SB)
                if sb + 2 < NSB:
                    nc.sync.dma_start(
                        hst_tiles[sb + 2],
                        hsT_d.ap()[:, (sb + 2) * SBW:(sb + 3) * SBW])
                hstb = hst_tiles[sb]

                qsq = [evp.tile([128, SB], bf16, tag=f"qsq{d}",
                                name=f"qsq{sb}_{d}") for d in range(DH)]
                MUL = mybir.AluOpType.mult

                def rope_combine(srcs, wtt, dstT, tag, beta=None):
                    # srcs[d]: raw projection half d ([128, SB], PSUM f32 or
                    # SBUF bf16)
                    ra = smp.tile([128, SB], bf16, tag="ra", bufs=3,
                                  name=f"ra{tag}{sb}")
                    rb = smp.tile([128, SB], bf16, tag="rb", bufs=3,
                                  name=f"rb{tag}{sb}")
                    nc.vector.scalar_tensor_tensor(
                        ra, srcs[0], wtt[0], cos_sb[0][:, sl], MUL, MUL)
                    nc.vector.scalar_tensor_tensor(
                        rb, srcs[1], wtt[1], sin_sb[0][:, sl], MUL, MUL)
                    if beta is None:
                        nc.vector.tensor_sub(dstT[0][:, sl], ra, rb)
                    else:
                        t0b = smp.tile([128, SB], bf16, tag="ra", bufs=3,
                                       name=f"t0{tag}{sb}")
                        nc.vector.tensor_sub(t0b, ra, rb)
                        nc.vector.tensor_mul(dstT[0][:, sl], t0b,
                                             beta[:, sl])
                    rc2 = smp.tile([128, SB], bf16, tag="ra", bufs=3,
                                   name=f"rc{tag}{sb}")
                    rd = smp.tile([128, SB], bf16, tag="rb", bufs=3,
                                  name=f"rd{tag}{sb}")
                    nc.vector.scalar_tensor_tensor(
                        rc2, srcs[1], wtt[1], cos_sb[1][:, sl], MUL, MUL)
                    nc.vector.scalar_tensor_tensor(
                        rd, srcs[0], wtt[0], sin_sb[1][:, sl], MUL, MUL)
                    if beta is None:
                        nc.vector.tensor_add(dstT[1][:, sl], rc2, rd)
                    else:
                        t1b = smp.tile([128, SB], bf16, tag="rb", bufs=3,
                                       name=f"t1{tag}{sb}")
                        nc.vector.tensor_add(t1b, rc2, rd)
                        nc.vector.tensor_mul(dstT[1][:, sl], t1b,
                                             beta[:, sl])

                # Q projection (both halves local) + rope-q
                qpps = []
                for d in range(DH):
                    pp = psA.tile([128, SB], f32, tag="acc",
                                  name=f"ppq{sb}_{d}")
                    for t in range(NHID):
                        lsl = slice(t * D + d * 128, t * D + (d + 1) * 128)
                        nc.tensor.matmul(pp, wq_sb[:, lsl],
                                         hstb[:, t * SB:(t + 1) * SB],
                                         start=(t == 0), stop=(t == NHID - 1))
                    nc.scalar.activation(qsq[d], pp, AF.Square)
                    qpps.append(pp)
                rope_combine([qpps[0][:, :], qpps[1][:, :]], wtq_sb, qwT, "q")

                # next s-block's K/V production + exchange (overlaps this
                # s-block's attention)
                if sb + 1 < NSB:
                    kv_produce(sb + 1)

                # previous s-block's o-proj: PE filler during the exchange
                if sb > 0:
                    for mm in range(4):
                        emit_oproj_block((sb - 1) * 4 + mm)

                # ---- alpha = 1/sqrt(mean(q^2)+eps), per query row ----
                ap_ps = psP.tile([128, 4], f32, tag="pp",
                                 name=f"aps{sb}")
                for m in range(4):
                    msl = slice(m * 128, (m + 1) * 128)
                    for d in range(DH):
                        nc.tensor.matmul(ap_ps[:, m:m + 1], qsq[d][:, msl],
                                         ones_sb, start=(d == 0),
                                         stop=(d == DH - 1))
                atmp = smp.tile([128, 4], f32, tag="atmp", name=f"atmp{sb}")
                nc.scalar.activation(atmp, ap_ps, AF.Sqrt,
                                     bias=epsq_sb, scale=1.0 / D)
                nc.vector.reciprocal_approx_fast(
                    alpha[:, sb * 4:(sb + 1) * 4], atmp)

                # ---- beta from the exchanged k^2 column sums ----
                kraw = kraw_t[sb]
                bps = bps_t[sb]
                bsum = smp.tile([1, SB], bf16, tag="bsum", name=f"bsum{sb}")
                nc.vector.tensor_add(bsum, bps[0], bps[1])
                btmp = smp.tile([1, SB], f32, tag="btmp", name=f"btmp{sb}")
                nc.scalar.activation(btmp, bsum, AF.Sqrt,
                                     bias=epsk_sb[0:1, :], scale=C2 / D)
                brow = smp.tile([1, SB], f32, tag="brow", name=f"brow{sb}")
                nc.vector.reciprocal_approx_fast(brow, btmp)
                browb = smp.tile([1, SB], bf16, tag="browb",
                                 name=f"browb{sb}")
                nc.vector.tensor_copy(browb, brow)
                nc.gpsimd.partition_broadcast(beta_bc[:, sl], browb)

                # ---- rope-k on the gathered raw halves ----
                rope_combine([kraw[0][:, :], kraw[1][:, :]], wtk_sb, kwT,
                             "k", beta=beta_bc)

                # ---- attention + o-proj per 128-row query block ----
                for m in range(4):
                    b = sb * 4 + m
                    q0 = b * 128
                    w = min(b + 1, 5) * 128
                    k0 = q0 + 128 - w
                    w1 = w - 128
                    qsl = slice(q0, q0 + 128)

                    if w1 > 0:
                        sc1 = psS1.tile([128, SB], f32, tag="sc1",
                                        name=f"sc1_{b}")
                        for d in range(DH):
                            nc.tensor.matmul(sc1[:, 0:w1], qwT[d][:, qsl],
                                             kwT[d][:, k0:k0 + w1],
                                             start=(d == 0),
                                             stop=(d == DH - 1))
                    sc2 = psT.tile([128, 128], f32, tag="tp",
                                   name=f"sc2_{b}")
                    for d in range(DH):
                        nc.tensor.matmul(sc2, qwT[d][:, qsl], kwT[d][:, qsl],
                                         start=(d == 0), stop=(d == DH - 1))

                    tt = smp.tile([128, MAXW], f32, tag="tanh", bufs=2,
                                  name=f"tt{b}")
                    if w1 > 0:
                        nc.scalar.activation(tt[:, 0:w1], sc1[:, 0:w1],
                                             AF.Tanh, scale=alpha[:, b:b + 1])
                    nc.scalar.activation(tt[:, w1:w], sc2, AF.Tanh,
                                         scale=alpha[:, b:b + 1])
                    nc.vector.tensor_add(tt[:, w1:w], tt[:, w1:w],
                                         mask_sb[:, 512:640])
                    if w == MAXW:
                        nc.vector.tensor_add(tt[:, 0:128], tt[:, 0:128],
                                             mask_sb[:, 0:128])

                    et = smp.tile([128, MAXW], bf16, tag="et", bufs=4,
                                  name=f"et{b}")
                    nc.scalar.activation(et[:, 0:w], tt[:, 0:w], AF.Exp,
                                         scale=SOFTCAP,
                                         accum_out=dn[:, b:b + 1])
                    # 1/denominator is applied at the o-proj eviction (rows
                    # of that PSUM are queries), keeping it off the softmax
                    # critical path
                    nc.vector.reciprocal_approx_fast(rc[:, b:b + 1],
                                                     dn[:, b:b + 1])

                    nchunks = w // 128
                    etcs = []
                    for c in range(nchunks):
                        tp = psT.tile([128, 128], bf16, tag="tp",
                                      name=f"tp{b}_{c}")
                        nc.tensor.transpose(tp, et[:, c * 128:(c + 1) * 128],
                                            id_sb)
                        etc = smp.tile([128, 128], bf16, tag="etc", bufs=6,
                                       name=f"etc{b}_{c}")
                        nc.vector.tensor_copy(etc, tp)
                        etcs.append(etc)
                    po = psP.tile([128, D], f32, tag="pp", name=f"po{b}")
                    for d in range(DH):
                        dsl = slice(d * 128, (d + 1) * 128)
                        for c in range(nchunks):
                            kvi = k0 // 128 + c
                            nc.tensor.matmul(po[:, dsl], v_sb[kvi][:, dsl],
                                             etcs[c], start=(c == 0),
                                             stop=(c == nchunks - 1))
                    for d in range(DH):
                        dsl = slice(d * 128, (d + 1) * 128)
                        nc.vector.tensor_copy(outT[d][:, qsl], po[:, dsl])

                    if sb == NSB - 1:
                        emit_oproj_block(b)

    nc.compile()
    return nc


def _prep_in_maps(hidden_states, position_ids, cos_table, sin_table,
                  Wq, Wk, Wv, Wo, q_norm_w, k_norm_w):
    hs = np.asarray(hidden_states, np.float32).reshape(S, HID)
    pos = np.asarray(position_ids).reshape(S).astype(np.int64)
    cos = np.asarray(cos_table, np.float32)[pos]   # [S, D]
    sin = np.asarray(sin_table, np.float32)[pos]
    Wq = np.asarray(Wq, np.float32)
    Wk = np.asarray(Wk, np.float32)
    Wv = np.asarray(Wv, np.float32)
    Wo = np.asarray(Wo, np.float32)

    # streamed layout: [128, sb*(NHID*SB) + t*SB + s'] so every DMA line is
    # wide and contiguous
    hsT = np.ascontiguousarray(
        hs.T.astype(BF16).reshape(NHID, 128, NSB, SB)
        .transpose(1, 2, 0, 3).reshape(128, NHID * S))
    cosT = np.ascontiguousarray(cos.T).astype(BF16)
    sinT = np.ascontiguousarray(sin.T).astype(BF16)

    def wtile(wslice):
        # [Dout, HID] -> [128, NHID*Dout] with chunk t at cols [t*Dout, ...)
        dout = wslice.shape[0]
        return np.ascontiguousarray(
            wslice.T.astype(BF16).reshape(NHID, 128, dout)
            .transpose(1, 0, 2).reshape(128, NHID * dout))
    wtq = (1.0 + np.asarray(q_norm_w, np.float32)).reshape(D, 1)
    wtk = (1.0 + np.asarray(k_norm_w, np.float32)).reshape(D, 1)

    i = np.arange(128)[:, None]
    j = np.arange(128)[None, :]
    mask = np.zeros((128, MAXW), np.float32)
    mask[:, 0:128] = np.where(j > i, 0.0, -2000.0)      # oldest chunk
    mask[:, 512:640] = np.where(j <= i, 0.0, -2000.0)   # causal chunk
    ident = np.eye(128, dtype=BF16)
    ones = np.ones((128, 1), BF16)

    in_maps = []
    for h in range(NCORES):
        kv = h // (H // KV)
        dh = h % 2
        qs = slice(h * D, (h + 1) * D)
        khs = slice(kv * D + dh * 128, kv * D + (dh + 1) * 128)
        in_maps.append({
            "hsT": hsT,
            "wqT": wtile(Wq[qs, :]),
            "wkT": wtile(Wk[khs, :]),
            "wvT": wtile(Wv[khs, :]),
            "woT": np.ascontiguousarray(Wo[:, qs].T).astype(BF16),
            "cosT": cosT, "sinT": sinT,
            "wtq": wtq, "wtk": wtk,
            "maskadd": mask, "ident": ident, "ones_": ones,
        })
    return in_maps


def kernel(hidden_states, position_ids, cos_table, sin_table,
           Wq, Wk, Wv, Wo, q_norm_w, k_norm_w):
    global _COMPILED, LAST_RESULT
    trace = bool(os.environ.get("BASS_TRACE"))
    if trace:
        _install_ntff_shim()
    from concourse import bass_utils

    if _COMPILED is None:
        _COMPILED = _build()

    in_maps = _prep_in_maps(hidden_states, position_ids, cos_table,
                            sin_table, Wq, Wk, Wv, Wo, q_norm_w, k_norm_w)
    res = bass_utils.run_bass_kernel_spmd(
        _COMPILED, in_maps, core_ids=list(range(NCORES)), trace=trace)
    LAST_RESULT = res

    out = res.results[0]["out"].astype(np.float32)
    for i in range(1, NCORES):
        out += res.results[i]["out"]
    return out.reshape(B, S, HID)


# revision 25
# speedup vs baseline: 1.1851x; 1.0243x over previous
"""Gemma3-style sliding-window attention on 8 Trainium2 NeuronCores.

Sharding: tensor-parallel over the 8 query heads (1 head per core, KV head
h//2 shared per pair). The K/V projections are split by dimension halves
across each core pair: core 2k computes dims [0,128) of its KV head's K and
V, core 2k+1 computes dims [128,256); the raw halves (plus the per-half
k^2 column sums) are exchanged with a pair-wise AllGather per s-block
(DRAM bounce, SWDGE so the bounce DMAs bypass the loaded HWDGE queues).
Each core then applies k-norm/RoPE locally on the gathered full-D raw K.
K/V for s-block j+1 are produced during iteration j so the exchange
latency hides behind local compute. A tiny warmup collective issued at
kernel start absorbs the ~45us ncfw entry latency.

Each core computes its head's partial o-proj output [S, HID]; the host
sums the 8 partials. All matmul operands are bf16 (rel-err budget 2e-2);
accumulation is f32 in PSUM.

Softmax trick: scores are softcapped by 50*tanh(.), so they are bounded in
[-50, 50] and exp() never overflows f32 -> no running-max subtraction.
Masking is additive (-2000) on the tanh output (pre-exp), which makes
masked exp() terms exactly 0.
"""

import os
import sys
import types

import numpy as np
import ml_dtypes

BF16 = ml_dtypes.bfloat16

B, S, HID = 1, 2048, 2560
H, KV, D = 8, 4, 256
SCALE = 256 ** -0.5
SOFTCAP = 50.0
WINDOW = 512
EPS = 1e-6
NCORES = 8
DH = 2                 # 128-partition halves of D
NHID = HID // 128      # 20
SB = 512               # s-block size
NSB = S // SB          # 4
NBLK = S // 128        # 16 query blocks
MAXW = WINDOW + 128    # max key span per query block

_COMPILED = None
LAST_RESULT = None     # BassKernelResults of the most recent run (for test.py)


def _install_ntff_shim():
    """The image's antenv lacks axon_hooks; recreate it so trace=True works."""
    try:
        from antenv import axon_hooks  # noqa: F401
        return
    except ImportError:
        pass
    try:
        import antenv
        import trn_agent_boot.trn_boot as tb

        hook = tb._ntff_profile_via_ctypes("/opt/axon/libaxon_pjrt.so")
        mod = types.ModuleType("antenv.axon_hooks")
        mod._hook = hook
        mod.get_axon_ntff_profile_hook = lambda: mod._hook
        mod.set_axon_ntff_profile_hook = lambda h: setattr(mod, "_hook", h)
        sys.modules["antenv.axon_hooks"] = mod
        antenv.axon_hooks = mod
    except Exception:
        pass


def _build():
    import concourse.mybir as mybir
    import concourse.tile as tile
    from concourse import bacc
    from concourse.mybir import ActivationFunctionType as AF

    f32 = mybir.dt.float32
    bf16 = mybir.dt.bfloat16

    nc = bacc.Bacc("TRN2", target_bir_lowering=False, debug=False,
                   num_devices=NCORES)

    hsT_d = nc.dram_tensor("hsT", [128, NHID * S], bf16, kind="ExternalInput")
    wqT_d = nc.dram_tensor("wqT", [128, NHID * D], bf16, kind="ExternalInput")
    # my d-half of the pair's K / V projection weights
    wkT_d = nc.dram_tensor("wkT", [128, NHID * 128], bf16,
                           kind="ExternalInput")
    wvT_d = nc.dram_tensor("wvT", [128, NHID * 128], bf16,
                           kind="ExternalInput")
    woT_d = nc.dram_tensor("woT", [D, HID], bf16, kind="ExternalInput")
    cosT_d = nc.dram_tensor("cosT", [D, S], bf16, kind="ExternalInput")
    sinT_d = nc.dram_tensor("sinT", [D, S], bf16, kind="ExternalInput")
    wtq_d = nc.dram_tensor("wtq", [D, 1], f32, kind="ExternalInput")
    wtk_d = nc.dram_tensor("wtk", [D, 1], f32, kind="ExternalInput")
    mask_d = nc.dram_tensor("maskadd", [128, MAXW], f32, kind="ExternalInput")
    id_d = nc.dram_tensor("ident", [128, 128], bf16, kind="ExternalInput")
    ones_d = nc.dram_tensor("ones_", [128, 1], bf16, kind="ExternalInput")
    out_d = nc.dram_tensor("out", [S, HID], bf16, kind="ExternalOutput")

    C2 = (SOFTCAP / SCALE) ** 2  # folds SCALE/SOFTCAP into the k-norm scale
    GROUPS = [[2 * k, 2 * k + 1] for k in range(NCORES // 2)]
    CCW = 1028     # bounce cols: [0:512] K, [512:1024] V, [1024:1028] bp

    with tile.TileContext(nc) as tc:
        with tc.tile_pool(name="const", bufs=1) as cp, \
             tc.tile_pool(name="hstp", bufs=3) as hstp, \
             tc.tile_pool(name="evp", bufs=3) as evp, \
             tc.tile_pool(name="smp", bufs=2) as smp, \
             tc.tile_pool(name="dram", bufs=2, space="DRAM") as dp, \
             tc.tile_pool(name="psA", bufs=3, space="PSUM") as psA, \
             tc.tile_pool(name="psS1", bufs=2, space="PSUM") as psS1, \
             tc.tile_pool(name="psT", bufs=2, space="PSUM") as psT, \
             tc.tile_pool(name="psP", bufs=1, space="PSUM") as psP:

            # ---- persistent constants ----
            wq_sb = cp.tile([128, NHID * D], bf16, tag="wq", name="wq")
            wk_sb = cp.tile([128, NHID * 128], bf16, tag="wk", name="wk")
            wv_sb = cp.tile([128, NHID * 128], bf16, tag="wv", name="wv")
            wo_sb = [cp.tile([128, HID], bf16, tag=f"wo{d}", name=f"wo{d}")
                     for d in range(DH)]
            cos_sb = [cp.tile([128, S], bf16, tag=f"cos{d}", name=f"cos{d}")
                      for d in range(DH)]
            sin_sb = [cp.tile([128, S], bf16, tag=f"sin{d}", name=f"sin{d}")
                      for d in range(DH)]
            wtq_sb = [cp.tile([128, 1], f32, tag=f"wtq{d}", name=f"wtq{d}")
                      for d in range(DH)]
            wtk_sb = [cp.tile([128, 1], f32, tag=f"wtk{d}", name=f"wtk{d}")
                      for d in range(DH)]
            mask_sb = cp.tile([128, MAXW], f32, tag="mask", name="mask")
            id_sb = cp.tile([128, 128], bf16, tag="ident", name="ident")
            ones_sb = cp.tile([128, 1], bf16, tag="ones", name="ones")
            epsq_sb = cp.tile([128, 1], f32, tag="epsq", name="epsq")
            epsk_sb = cp.tile([1, 1], f32, tag="epsk", name="epsk")
            nc.vector.memset(epsq_sb, EPS)
            nc.vector.memset(epsk_sb, C2 * EPS)

            # persistent activations
            qwT = [cp.tile([128, S], bf16, tag=f"qwT{d}", name=f"qwT{d}")
                   for d in range(DH)]
            kwT = [cp.tile([128, S], bf16, tag=f"kwT{d}", name=f"kwT{d}")
                   for d in range(DH)]
            outT = [cp.tile([128, S], bf16, tag=f"outT{d}", name=f"outT{d}")
                    for d in range(DH)]
            v_sb = [cp.tile([128, D], bf16, tag=f"v{m}", name=f"v{m}")
                    for m in range(NBLK)]
            beta_bc = cp.tile([128, S], bf16, tag="betabc", name="betabc")
            alpha = cp.tile([128, NBLK], f32, tag="alpha", name="alpha")
            dn = cp.tile([128, NBLK], f32, tag="dn", name="dn")
            rc = cp.tile([128, NBLK], f32, tag="rc", name="rc")

            # warmup collective: absorbs the ~45us ncfw entry latency while
            # the input DMAs stream. Gathers garbage (wu_in is never
            # written) so it has no input dependency and triggers at t~0.
            wu_in = dp.tile([1, 16], bf16, tag="wui", name="wu_in")
            wu_out = dp.tile([2, 16], bf16, tag="wuo", name="wu_out")
            nc.gpsimd.collective_compute(
                "AllGather", mybir.AluOpType.bypass, replica_groups=GROUPS,
                ins=[wu_in.opt()], outs=[wu_out.opt()])

            # piecewise wide-line loads, interleaved so the first
            # projection matmuls start after the first piece lands
            SBW = NHID * SB      # columns per s-block in the streamed layout
            hst_tiles = [hstp.tile([128, SBW], bf16, tag="hstbig",
                                   name=f"hst_sb{j}") for j in range(NSB)]
            pieces = [1, 2, 2, 5, 5, 5]   # chunk counts; small first pieces
            c0 = 0
            for cnt in pieces:
                wsl = slice(c0 * D, (c0 + cnt) * D)
                hsl2 = slice(c0 * 128, (c0 + cnt) * 128)
                nc.sync.dma_start(wk_sb[:, hsl2], wkT_d.ap()[:, hsl2])
                nc.sync.dma_start(wq_sb[:, wsl], wqT_d.ap()[:, wsl])
                hsl = slice(c0 * SB, (c0 + cnt) * SB)
                nc.sync.dma_start(hst_tiles[0][:, hsl], hsT_d.ap()[:, hsl])
                c0 += cnt
            nc.sync.dma_start(wv_sb, wvT_d.ap())
            nc.sync.dma_start(hst_tiles[1], hsT_d.ap()[:, SBW:2 * SBW])
            # cos/sin for s-block 0 first (rope-q of sb0), rest after hst1
            for d in range(DH):
                r = slice(d * 128, (d + 1) * 128)
                nc.sync.dma_start(cos_sb[d][:, 0:SB], cosT_d.ap()[r, 0:SB])
                nc.sync.dma_start(sin_sb[d][:, 0:SB], sinT_d.ap()[r, 0:SB])
                nc.sync.dma_start(wtq_sb[d], wtq_d.ap()[r, :])
                nc.sync.dma_start(wtk_sb[d], wtk_d.ap()[r, :])
            nc.sync.dma_start(mask_sb, mask_d.ap())
            nc.sync.dma_start(id_sb, id_d.ap())
            nc.sync.dma_start(ones_sb, ones_d.ap())
            for d in range(DH):
                r = slice(d * 128, (d + 1) * 128)
                nc.sync.dma_start(cos_sb[d][:, SB:], cosT_d.ap()[r, SB:])
                nc.sync.dma_start(sin_sb[d][:, SB:], sinT_d.ap()[r, SB:])
            # needed only at o-proj
            for d in range(DH):
                r = slice(d * 128, (d + 1) * 128)
                nc.sync.dma_start(wo_sb[d], woT_d.ap()[r, :])

            def emit_oproj_block(b):
                    qsl = slice(b * 128, (b + 1) * 128)
                    orow = smp.tile([128, HID], bf16, tag="orow", bufs=2,
                                    name=f"orow{b}")
                    for n in range(5):
                        op = psA.tile([128, SB], f32, tag="acc",
                                      name=f"op{b}_{n}")
                        nsl = slice(n * 512, (n + 1) * 512)
                        for d in range(DH):
                            nc.tensor.matmul(op, outT[d][:, qsl],
                                             wo_sb[d][:, nsl],
                                             start=(d == 0),
                                             stop=(d == DH - 1))
                        if n % 2 == 0:
                            nc.scalar.mul(orow[:, nsl], op, rc[:, b:b + 1])
                        else:
                            nc.vector.tensor_scalar_mul(orow[:, nsl], op,
                                                        rc[:, b:b + 1])
                        if b == NBLK - 1:
                            # last block: per-chunk DMA so the tail drains
                            # as each eviction completes
                            nc.sync.dma_start(
                                out_d.ap()[b * 128:(b + 1) * 128, nsl],
                                orow[:, nsl])
                    if b != NBLK - 1:
                        nc.sync.dma_start(
                            out_d.ap()[b * 128:(b + 1) * 128, :], orow)

            # per-s-block exchange state, filled by kv_produce(j)
            kraw_t = [None] * NSB
            bps_t = [None] * NSB

            def kv_produce(j):
                """Project my K/V d-half of s-block j, issue the pair
                AllGather, and the readbacks into kraw/v_sb/bps."""
                hstj = hst_tiles[j]
                kp = psA.tile([128, SB], f32, tag="acc", name=f"kp{j}")
                for t in range(NHID):
                    nc.tensor.matmul(kp, wk_sb[:, t * 128:(t + 1) * 128],
                                     hstj[:, t * SB:(t + 1) * SB],
                                     start=(t == 0), stop=(t == NHID - 1))
                kraw_my = smp.tile([128, SB], bf16, tag="krawm", bufs=2,
                                   name=f"krawm{j}")
                nc.vector.tensor_copy(kraw_my, kp)
                ksq_my = smp.tile([128, SB], bf16, tag="ksqm", bufs=1,
                                  name=f"ksqm{j}")
                nc.vector.tensor_mul(ksq_my, kraw_my, kraw_my)
                bpp = psS1.tile([1, SB], f32, tag="sc1", name=f"bpp{j}")
                nc.tensor.matmul(bpp, ones_sb, ksq_my, start=True, stop=True)
                bp_my = smp.tile([1, SB], bf16, tag="bpm", bufs=2,
                                 name=f"bpm{j}")
                nc.vector.tensor_copy(bp_my, bpp)

                vraw_my = []
                for m in range(4):
                    vp = psP.tile([128, 128], f32, tag="pp",
                                  name=f"vp{j}_{m}")
                    for t in range(NHID):
                        msl = slice(t * SB + m * 128, t * SB + (m + 1) * 128)
                        nc.tensor.matmul(vp, hstj[:, msl],
                                         wv_sb[:, t * 128:(t + 1) * 128],
                                         start=(t == 0), stop=(t == NHID - 1))
                    vr = smp.tile([128, 128], bf16, tag="vraw", bufs=4,
                                  name=f"vraw{j}_{m}")
                    nc.vector.tensor_copy(vr, vp)
                    vraw_my.append(vr)

                cc_in = dp.tile([128, CCW], bf16, tag="ccin", name=f"cci{j}")
                cc_out = dp.tile([256, CCW], bf16, tag="ccout",
                                 name=f"cco{j}")
                nc.gpsimd.dma_start(cc_in[:, 0:SB], kraw_my)
                for m in range(4):
                    nc.gpsimd.dma_start(
                        cc_in[:, SB + m * 128:SB + (m + 1) * 128],
                        vraw_my[m])
                nc.gpsimd.dma_start(cc_in[:, 1024:1028], bp_my)
                nc.gpsimd.collective_compute(
                    "AllGather", mybir.AluOpType.bypass,
                    replica_groups=GROUPS,
                    ins=[cc_in.opt()], outs=[cc_out.opt()])
                kraw = [smp.tile([128, SB], bf16, tag=f"kraw{d}", bufs=2,
                                 name=f"kraw{j}_{d}") for d in range(DH)]
                nc.sync.dma_start(kraw[0], cc_out[0:128, 0:SB])
                nc.sync.dma_start(kraw[1], cc_out[128:256, 0:SB])
                for m in range(4):
                    csl = slice(SB + m * 128, SB + (m + 1) * 128)
                    nc.sync.dma_start(v_sb[j * 4 + m][:, 0:128],
                                      cc_out[0:128, csl])
                    nc.sync.dma_start(v_sb[j * 4 + m][:, 128:256],
                                      cc_out[128:256, csl])
                bps = [smp.tile([1, SB], bf16, tag=f"bps{d}", bufs=2,
                                name=f"bps{j}_{d}") for d in range(DH)]
                nc.sync.dma_start(bps[0], cc_out[0:128, 1024:1028])
                nc.sync.dma_start(bps[1], cc_out[128:256, 1024:1028])
                kraw_t[j] = kraw
                bps_t[j] = bps

            kv_produce(0)

            for sb in range(NSB):
                s0 = sb * SB
                sl = slice(s0, s0 + SB)
                if sb + 2 < NSB:
                    nc.sync.dma_start(
                        hst_tiles[sb + 2],
                        hsT_d.ap()[:, (sb + 2) * SBW:(sb + 3) * SBW])
                hstb = hst_tiles[sb]

                qsq = [evp.tile([128, SB], bf16, tag=f"qsq{d}",
                                name=f"qsq{sb}_{d}") for d in range(DH)]
                MUL = mybir.AluOpType.mult

                def rope_combine(srcs, wtt, dstT, tag, beta=None):
                    # srcs[d]: raw projection half d ([128, SB], PSUM f32 or
                    # SBUF bf16)
                    ra = smp.tile([128, SB], bf16, tag="ra", bufs=3,
                                  name=f"ra{tag}{sb}")
                    rb = smp.tile([128, SB], bf16, tag="rb", bufs=3,
                                  name=f"rb{tag}{sb}")
                    nc.vector.scalar_tensor_tensor(
                        ra, srcs[0], wtt[0], cos_sb[0][:, sl], MUL, MUL)
                    nc.vector.scalar_tensor_tensor(
                        rb, srcs[1], wtt[1], sin_sb[0][:, sl], MUL, MUL)
                    if beta is None:
                        nc.vector.tensor_sub(dstT[0][:, sl], ra, rb)
                    else:
                        t0b = smp.tile([128, SB], bf16, tag="ra", bufs=3,
                                       name=f"t0{tag}{sb}")
                        nc.vector.tensor_sub(t0b, ra, rb)
                        nc.vector.tensor_mul(dstT[0][:, sl], t0b,
                                             beta[:, sl])
                    rc2 = smp.tile([128, SB], bf16, tag="ra", bufs=3,
                                   name=f"rc{tag}{sb}")
                    rd = smp.tile([128, SB], bf16, tag="rb", bufs=3,
                                  name=f"rd{tag}{sb}")
                    nc.vector.scalar_tensor_tensor(
                        rc2, srcs[1], wtt[1], cos_sb[1][:, sl], MUL, MUL)
                    nc.vector.scalar_tensor_tensor(
                        rd, srcs[0], wtt[0], sin_sb[1][:, sl], MUL, MUL)
                    if beta is None:
                        nc.vector.tensor_add(dstT[1][:, sl], rc2, rd)
                    else:
                        t1b = smp.tile([128, SB], bf16, tag="rb", bufs=3,
                                       name=f"t1{tag}{sb}")
                        nc.vector.tensor_add(t1b, rc2, rd)
                        nc.vector.tensor_mul(dstT[1][:, sl], t1b,
                                             beta[:, sl])

                # Q projection (both halves local) + rope-q
                qraw = []
                for d in range(DH):
                    pp = psA.tile([128, SB], f32, tag="acc",
                                  name=f"ppq{sb}_{d}")
                    for t in range(NHID):
                        lsl = slice(t * D + d * 128, t * D + (d + 1) * 128)
                        nc.tensor.matmul(pp, wq_sb[:, lsl],
                                         hstb[:, t * SB:(t + 1) * SB],
                                         start=(t == 0), stop=(t == NHID - 1))
                    qr = smp.tile([128, SB], bf16, tag=f"qraw{d}", bufs=2,
                                  name=f"qraw{sb}_{d}")
                    nc.vector.tensor_copy(qr, pp)
                    nc.vector.tensor_mul(qsq[d], qr, qr)
                    qraw.append(qr)
                rope_combine([qraw[0][:, :], qraw[1][:, :]], wtq_sb, qwT, "q")

                # next s-block's K/V production + exchange (overlaps this
                # s-block's attention)
                if sb + 1 < NSB:
                    kv_produce(sb + 1)

                # previous s-block's o-proj: PE filler during the exchange
                if sb > 0:
                    for mm in range(4):
                        emit_oproj_block((sb - 1) * 4 + mm)

                # ---- alpha = 1/sqrt(mean(q^2)+eps), per query row ----
                ap_ps = psP.tile([128, 4], f32, tag="pp",
                                 name=f"aps{sb}")
                for m in range(4):
                    msl = slice(m * 128, (m + 1) * 128)
                    for d in range(DH):
                        nc.tensor.matmul(ap_ps[:, m:m + 1], qsq[d][:, msl],
                                         ones_sb, start=(d == 0),
                                         stop=(d == DH - 1))
                atmp = smp.tile([128, 4], f32, tag="atmp", name=f"atmp{sb}")
                nc.scalar.activation(atmp, ap_ps, AF.Sqrt,
                                     bias=epsq_sb, scale=1.0 / D)
                nc.vector.reciprocal_approx_fast(
                    alpha[:, sb * 4:(sb + 1) * 4], atmp)

                # ---- beta from the exchanged k^2 column sums ----
                kraw = kraw_t[sb]
                bps = bps_t[sb]
                bsum = smp.tile([1, SB], bf16, tag="bsum", name=f"bsum{sb}")
                nc.vector.tensor_add(bsum, bps[0], bps[1])
                btmp = smp.tile([1, SB], f32, tag="btmp", name=f"btmp{sb}")
                nc.scalar.activation(btmp, bsum, AF.Sqrt,
                                     bias=epsk_sb[0:1, :], scale=C2 / D)
                brow = smp.tile([1, SB], f32, tag="brow", name=f"brow{sb}")
                nc.vector.reciprocal_approx_fast(brow, btmp)
                browb = smp.tile([1, SB], bf16, tag="browb",
                                 name=f"browb{sb}")
                nc.vector.tensor_copy(browb, brow)
                nc.gpsimd.partition_broadcast(beta_bc[:, sl], browb)

                # ---- rope-k on the gathered raw halves ----
                rope_combine([kraw[0][:, :], kraw[1][:, :]], wtk_sb, kwT,
                             "k", beta=beta_bc)

                # ---- attention + o-proj per 128-row query block ----
                for m in range(4):
                    b = sb * 4 + m
                    q0 = b * 128
                    w = min(b + 1, 5) * 128
                    k0 = q0 + 128 - w
                    w1 = w - 128
                    qsl = slice(q0, q0 + 128)

                    if w1 > 0:
                        sc1 = psS1.tile([128, SB], f32, tag="sc1",
                                        name=f"sc1_{b}")
                        for d in range(DH):
                            nc.tensor.matmul(sc1[:, 0:w1], qwT[d][:, qsl],
                                             kwT[d][:, k0:k0 + w1],
                                             start=(d == 0),
                                             stop=(d == DH - 1))
                    sc2 = psT.tile([128, 128], f32, tag="tp",
                                   name=f"sc2_{b}")
                    for d in range(DH):
                        nc.tensor.matmul(sc2, qwT[d][:, qsl], kwT[d][:, qsl],
                                         start=(d == 0), stop=(d == DH - 1))

                    tt = smp.tile([128, MAXW], f32, tag="tanh", bufs=2,
                                  name=f"tt{b}")
                    if w1 > 0:
                        nc.scalar.activation(tt[:, 0:w1], sc1[:, 0:w1],
                                             AF.Tanh, scale=alpha[:, b:b + 1])
                    nc.scalar.activation(tt[:, w1:w], sc2, AF.Tanh,
                                         scale=alpha[:, b:b + 1])
                    nc.vector.tensor_add(tt[:, w1:w], tt[:, w1:w],
                                         mask_sb[:, 512:640])
                    if w == MAXW:
                        nc.vector.tensor_add(tt[:, 0:128], tt[:, 0:128],
                                             mask_sb[:, 0:128])

                    et = smp.tile([128, MAXW], bf16, tag="et", bufs=3,
                                  name=f"et{b}")
                    nc.scalar.activation(et[:, 0:w], tt[:, 0:w], AF.Exp,
                                         scale=SOFTCAP,
                                         accum_out=dn[:, b:b + 1])
                    # 1/denominator is applied at the o-proj eviction (rows
                    # of that PSUM are queries), keeping it off the softmax
                    # critical path
                    nc.vector.reciprocal_approx_fast(rc[:, b:b + 1],
                                                     dn[:, b:b + 1])

                    nchunks = w // 128
                    etcs = []
                    for c in range(nchunks):
                        tp = psT.tile([128, 128], bf16, tag="tp",
                                      name=f"tp{b}_{c}")
                        nc.tensor.transpose(tp, et[:, c * 128:(c + 1) * 128],
                                            id_sb)
                        etc = smp.tile([128, 128], bf16, tag="etc", bufs=6,
                                       name=f"etc{b}_{c}")
                        nc.vector.tensor_copy(etc, tp)
                        etcs.append(etc)
                    po = psP.tile([128, D], f32, tag="pp", name=f"po{b}")
                    for d in range(DH):
                        dsl = slice(d * 128, (d + 1) * 128)
                        for c in range(nchunks):
                            kvi = k0 // 128 + c
                            nc.tensor.matmul(po[:, dsl], v_sb[kvi][:, dsl],
                                             etcs[c], start=(c == 0),
                                             stop=(c == nchunks - 1))
                    for d in range(DH):
                        dsl = slice(d * 128, (d + 1) * 128)
                        nc.vector.tensor_copy(outT[d][:, qsl], po[:, dsl])

                    if sb == NSB - 1:
                        emit_oproj_block(b)

    nc.compile()
    return nc


def _prep_in_maps(hidden_states, position_ids, cos_table, sin_table,
                  Wq, Wk, Wv, Wo, q_norm_w, k_norm_w):
    hs = np.asarray(hidden_states, np.float32).reshape(S, HID)
    pos = np.asarray(position_ids).reshape(S).astype(np.int64)
    cos = np.asarray(cos_table, np.float32)[pos]   # [S, D]
    sin = np.asarray(sin_table, np.float32)[pos]
    Wq = np.asarray(Wq, np.float32)
    Wk = np.asarray(Wk, np.float32)
    Wv = np.asarray(Wv, np.float32)
    Wo = np.asarray(Wo, np.float32)

    # streamed layout: [128, sb*(NHID*SB) + t*SB + s'] so every DMA line is
    # wide and contiguous
    hsT = np.ascontiguousarray(
        hs.T.astype(BF16).reshape(NHID, 128, NSB, SB)
        .transpose(1, 2, 0, 3).reshape(128, NHID * S))
    cosT = np.ascontiguousarray(cos.T).astype(BF16)
    sinT = np.ascontiguousarray(sin.T).astype(BF16)

    def wtile(wslice):
        # [Dout, HID] -> [128, NHID*Dout] with chunk t at cols [t*Dout, ...)
        dout = wslice.shape[0]
        return np.ascontiguousarray(
            wslice.T.astype(BF16).reshape(NHID, 128, dout)
            .transpose(1, 0, 2).reshape(128, NHID * dout))
    wtq = (1.0 + np.asarray(q_norm_w, np.float32)).reshape(D, 1)
    wtk = (1.0 + np.asarray(k_norm_w, np.float32)).reshape(D, 1)

    i = np.arange(128)[:, None]
    j = np.arange(128)[None, :]
    mask = np.zeros((128, MAXW), np.float32)
    mask[:, 0:128] = np.where(j > i, 0.0, -2000.0)      # oldest chunk
    mask[:, 512:640] = np.where(j <= i, 0.0, -2000.0)   # causal chunk
    ident = np.eye(128, dtype=BF16)
    ones = np.ones((128, 1), BF16)

    in_maps = []
    for h in range(NCORES):
        kv = h // (H // KV)
        dh = h % 2
        qs = slice(h * D, (h + 1) * D)
        khs = slice(kv * D + dh * 128, kv * D + (dh + 1) * 128)
        in_maps.append({
            "hsT": hsT,
            "wqT": wtile(Wq[qs, :]),
            "wkT": wtile(Wk[khs, :]),
            "wvT": wtile(Wv[khs, :]),
            "woT": np.ascontiguousarray(Wo[:, qs].T).astype(BF16),
            "cosT": cosT, "sinT": sinT,
            "wtq": wtq, "wtk": wtk,
            "maskadd": mask, "ident": ident, "ones_": ones,
        })
    return in_maps


def kernel(hidden_states, position_ids, cos_table, sin_table,
           Wq, Wk, Wv, Wo, q_norm_w, k_norm_w):
    global _COMPILED, LAST_RESULT
    trace = bool(os.environ.get("BASS_TRACE"))
    if trace:
        _install_ntff_shim()
    from concourse import bass_utils

    if _COMPILED is None:
        _COMPILED = _build()

    in_maps = _prep_in_maps(hidden_states, position_ids, cos_table,
                            sin_table, Wq, Wk, Wv, Wo, q_norm_w, k_norm_w)
    res = bass_utils.run_bass_kernel_spmd(
        _COMPILED, in_maps, core_ids=list(range(NCORES)), trace=trace)
    LAST_RESULT = res

    out = res.results[0]["out"].astype(np.float32)
    for i in range(1, NCORES):
        out += res.results[i]["out"]
    return out.reshape(B, S, HID)
